# revision 14
# baseline (speedup 1.0000x reference)
"""MiniMHSA Trainium2 kernel: 8 NeuronCores, shard = (batch n, head-group).

Reference computes, per batch n:
  qkv = x @ W_qkv.T + b_qkv ; split into q,k,v heads (H=16, HD=64)
  scores = (q @ k.T) / sqrt(HD), masked keys -> -1e9, softmax, @ v
  out = attn_out @ W_out.T + b_out

Core c handles n = c//2 and head-group hg = c%2 (8 heads each).

Key ideas vs the naive version:
  * Mask compaction: masked keys get exp(-1e9)=0 exactly, so the host
    gathers only the valid keys (~half) into xkT[D, Lv]; k/v projection,
    scores, exp and AV all shrink by ~2x. Pad keys carry bias -1e9.
  * bf16 operands everywhere on the PE (same 1 cycle/row as f32r, half
    the SBUF/DMA), fp32 PSUM accumulation throughout.
  * Single pass over all 8 heads; attention output otn kept in SBUF as
    head PAIRS on 128 partitions (even head rows 0:64, odd rows 64:128)
    so the out-projection contracts 128-deep. Odd heads put the softmax
    denominator at PSUM row 63 by placing the ones column FIRST in the
    stationary v tile.
  * Software-pipelined emission: q-proj of hemi 1 rides inside hemi-0
    attention; out-proj of hemi 0 rides inside hemi-1 attention. The
    softmax exp (scalar engine, the true bottleneck) never waits.
  * y is DMAed straight from PSUM (no DVE copy).
"""
import sys

sys.path.insert(0, '/opt/trn_rl_repo')


import numpy as np

_KERNEL_CACHE = {}


def _split_excess_waits(nc):
    """Walrus codegen reliably accepts only ONE sync wait per instruction
    (Matmult hard-fails at 2, Drain at 5). Tile's scheduler can attach more.
    Move excess waits onto preceding same-engine NOPs — semantically identical
    since engine queues execute in order."""
    from concourse import mybir

    for f in nc.m.functions:
        for blk in f.blocks:
            il = blk.instructions
            i = 0
            while i < len(il):
                inst = il[i]
                si = inst.sync_info
                waits = list(si.on_wait) if si is not None and si.on_wait else []
                if len(waits) > 1:
                    keep = waits[-1:]
                    excess = waits[:-1]
                    pos = i
                    for j, wcond in enumerate(excess):
                        nop = mybir.InstNoOp(name=f"{inst.name}-ws{j}", ins=[], outs=[])
                        nop.engine = inst.engine
                        nop.sync_info = mybir.SyncInfo(on_wait=[wcond], on_update=[])
                        il.insert(pos, nop)
                        pos += 1
                        i += 1
                    inst.sync_info = mybir.SyncInfo(
                        on_wait=keep,
                        on_update=list(si.on_update) if si.on_update else [],
                    )
                i += 1


def _build(cfg, waitsplit=True):
    import concourse.bass as bass
    import concourse.tile as tile
    from concourse import mybir

    F32 = mybir.dt.float32
    F32R = mybir.dt.float32r
    BF16 = mybir.dt.bfloat16
    AF = mybir.ActivationFunctionType
    MULT = mybir.AluOpType.mult

    L, D, HC, HD = cfg["L"], cfg["D"], cfg["HC"], cfg["HD"]
    Lv = cfg["Lv"]            # padded valid-key count (multiple of 128)
    DCH = D // 128            # contraction chunks for projections
    DV = HC * HD              # qkv width per core (512)
    MC = DV // 128            # m-chunks for q (and for k) = head pairs = 4
    KC = Lv // 128            # attention key chunks
    QH = L // 1024            # attention q hemis (1024 wide)
    DOUT = D
    DC = DOUT // 512

    nc = bass.Bass()
    xT_d = nc.dram_tensor("xT", [D, L], BF16, kind="ExternalInput")
    xkT_d = nc.dram_tensor("xkT", [D, Lv], BF16, kind="ExternalInput")
    wqk_d = nc.dram_tensor("wqk", [128, DCH, 2 * DV], BF16, kind="ExternalInput")
    wv_d = nc.dram_tensor("wv", [128, DCH, DV], BF16, kind="ExternalInput")
    bqk_d = nc.dram_tensor("bqk", [128, 2 * MC], F32, kind="ExternalInput")
    bv_d = nc.dram_tensor("bv", [1, DV], F32, kind="ExternalInput")
    mb_d = nc.dram_tensor("mb", [128, KC], F32, kind="ExternalInput")
    wo_d = nc.dram_tensor("wo", [128, MC, DOUT], BF16, kind="ExternalInput")
    bo_d = nc.dram_tensor("bo", [1, DOUT], F32, kind="ExternalInput")
    y_d = nc.dram_tensor("y", [L, DOUT], BF16, kind="ExternalOutput")

    # k/v projections consume compacted keys in chunks of <=512 columns
    kv_chunks = []
    off = 0
    while off < Lv:
        w = min(512, Lv - off)
        kv_chunks.append((off, w))
        off += w

    with tile.TileContext(nc) as tc, \
         nc.allow_low_precision(reason="bf16 matmuls intended"):
        with tc.tile_pool(name="const", bufs=1) as const, \
             tc.tile_pool(name="wpool", bufs=1) as wpool, \
             tc.tile_pool(name="big", bufs=1) as big, \
             tc.tile_pool(name="xkpool", bufs=2) as xkpool, \
             tc.tile_pool(name="xtpool", bufs=2) as xtpool, \
             tc.tile_pool(name="workP", bufs=12) as workP, \
             tc.tile_pool(name="workS", bufs=2) as workS, \
             tc.tile_pool(name="psB", bufs=2, space="PSUM") as psB, \
             tc.tile_pool(name="psC", bufs=2, space="PSUM") as psC:

            # ---- weight/const DMAs (pool queue), x DMAs (sync queue) ----
            wqk_r = wpool.tile([128, DCH, 2 * DV], BF16)
            # k half first: kv projection starts as soon as possible
            nc.gpsimd.dma_start(out=wqk_r[:, :, DV:2 * DV], in_=wqk_d[:, :, DV:2 * DV])
            wv_r = wpool.tile([128, DCH, DV], BF16)
            nc.gpsimd.dma_start(out=wv_r, in_=wv_d[:, :, :])
            nc.gpsimd.dma_start(out=wqk_r[:, :, 0:DV], in_=wqk_d[:, :, 0:DV])
            bqk_t = const.tile([128, 2 * MC], F32)
            nc.gpsimd.dma_start(out=bqk_t, in_=bqk_d[:, :])
            mb_t = const.tile([128, KC], F32)
            nc.gpsimd.dma_start(out=mb_t, in_=mb_d[:, :])
            bv_r = const.tile([1, DV], F32R)
            nc.gpsimd.dma_start(out=bv_r, in_=bv_d[:, :])
            wo_r = wpool.tile([128, MC, DOUT], BF16)
            nc.gpsimd.dma_start(out=wo_r, in_=wo_d[:, :, :])
            bo_r = const.tile([1, DOUT], F32R)
            nc.gpsimd.dma_start(out=bo_r, in_=bo_d[:, :])

            ones_f = const.tile([128, 1], F32)
            nc.vector.memset(ones_f, 1.0)
            ones_r = const.tile([1, 128], F32R)
            nc.vector.tensor_copy(out=ones_r, in_=ones_f[0:1, 0:1].broadcast_to([1, 128]))

            # ---- persistent SBUF state ----
            qT = big.tile([128, MC, L], BF16, tag="qT")
            kT = big.tile([128, MC, Lv], BF16, tag="kT")
            # vp cols: 0..63 = v, 64 = ones (softmax denominator row)
            vp = big.tile([128, KC, HC, HD + 1], BF16, tag="vp")
            nc.vector.memset(vp[:, :, :, HD:HD + 1], 1.0)
            otn = big.tile([128, MC, L], BF16, tag="otn")

            # ---------------- k/v projection (compacted keys) --------------
            with tc.tile_pool(name="psA", bufs=2, space="PSUM") as psA:
                for (off, w) in kv_chunks:
                    xkt = xkpool.tile([128, DCH, 512], BF16)
                    nc.sync.dma_start(
                        out=xkt[:, :, 0:w],
                        in_=xkT_d.rearrange("(c p) l -> p c l", p=128)[:, :, off:off + w],
                    )
                    for mc in range(MC):
                        k_ps = psA.tile([128, 512], F32, tag="pa")
                        for k in range(DCH):
                            nc.tensor.matmul(
                                k_ps[:, 0:w],
                                wqk_r[:, k, DV + mc * 128:DV + (mc + 1) * 128],
                                xkt[:, k, 0:w],
                                start=(k == 0), stop=(k == DCH - 1),
                            )
                        nc.vector.tensor_scalar_add(
                            out=kT[:, mc, off:off + w],
                            in0=k_ps[:, 0:w], scalar1=bqk_t[:, MC + mc:MC + mc + 1],
                        )
                    for sub in range(w // 128):
                        kcg = (off + sub * 128) // 128
                        v_ps = psA.tile([128, DV], F32, tag="pa")
                        for k in range(DCH):
                            nc.tensor.matmul(
                                v_ps[:, :],
                                xkt[:, k, sub * 128:(sub + 1) * 128],
                                wv_r[:, k, :],
                                start=(k == 0), stop=False,
                            )
                        nc.tensor.matmul(
                            v_ps[:, :], ones_r[0:1, :], bv_r[0:1, :],
                            start=False, stop=True,
                        )
                        nc.vector.tensor_copy(
                            out=vp[:, kcg, :, 0:HD],
                            in_=v_ps.rearrange("p (h d) -> p h d", h=HC),
                        )

                # ------------- q projection, hemi 0 ------------------------
                def emit_qproj(lc):
                    xt = xtpool.tile([128, DCH, 512], BF16)
                    nc.sync.dma_start(
                        out=xt,
                        in_=xT_d.rearrange("(c p) l -> p c l", p=128)[:, :, lc * 512:(lc + 1) * 512],
                    )
                    for mc in range(MC):
                        q_ps = psA.tile([128, 512], F32, tag="pa")
                        for k in range(DCH):
                            nc.tensor.matmul(
                                q_ps[:, :],
                                wqk_r[:, k, mc * 128:(mc + 1) * 128],
                                xt[:, k, :],
                                start=(k == 0), stop=(k == DCH - 1),
                            )
                        nc.vector.tensor_scalar_add(
                            out=qT[:, mc, lc * 512:(lc + 1) * 512],
                            in0=q_ps, scalar1=bqk_t[:, mc:mc + 1],
                        )

                for lc in range(2):
                    emit_qproj(lc)

                # ------------- attention hemi 0 (+ q proj hemi 1) ----------
                # Normalize (recip -> bc -> copy -> mult) is emitted one AV
                # block LATE: by the time its bc matmul reaches the PE queue,
                # the DVE reciprocal finished long ago — no PE stall.
                pending_norm = []

                def flush_norm(n=1):
                    for _ in range(n if n >= 0 else len(pending_norm)):
                        if not pending_norm:
                            return
                        pending_norm.pop(0)()

                def emit_attention_head(h, qh):
                    """scores+exp for all kc, then AV per 512-wide qq; the
                    normalize of each AV block is queued on pending_norm."""
                    c, par = h // 2, h % 2
                    q0 = qh * 1024
                    pts = []
                    for kc in range(KC):
                        st = psB.tile([128, 1024], F32, tag="st")
                        for s in range(2):
                            nc.tensor.matmul(
                                st[:, s * 512:(s + 1) * 512],
                                kT[64 * par:64 * par + 64, c, kc * 128:(kc + 1) * 128],
                                qT[64 * par:64 * par + 64, c, q0 + s * 512:q0 + (s + 1) * 512],
                                start=True, stop=True,
                            )
                        pt = workP.tile([128, 1024], BF16, tag="pT")
                        nc.scalar.activation(
                            out=pt, in_=st, func=AF.Exp,
                            bias=mb_t[:, kc:kc + 1], scale=1.0,
                        )
                        pts.append(pt)
                    for qq in range(2):
                        # AV: v rows 0..63, denominator row 64 for all heads.
                        # Odd heads land in otn rows 64:128 via a 64-partition
                        # shift on the normalize ops (bases stay 32-aligned).
                        ot = psC.tile([128, 512], F32, tag="ot")
                        orows = (0, HD) if par == 0 else (64, 128)
                        for kc in range(KC):
                            nc.tensor.matmul(
                                ot[0:HD + 1, :],
                                vp[:, kc, h, :],
                                pts[kc][:, qq * 512:(qq + 1) * 512],
                                start=(kc == 0), stop=(kc == KC - 1),
                            )
                        recip = workS.tile([1, 512], F32R, tag="recip")
                        nc.vector.reciprocal(out=recip, in_=ot[HD:HD + 1, :])

                        def norm(ot=ot, recip=recip, orows=orows, c=c, q0=q0, qq=qq):
                            # PE dst must sit at partition 0 — broadcast the
                            # reciprocal to all 128 partitions (same column
                            # count); each head reads the 64-row half it needs.
                            bc_ps = psB.tile([128, 1024], F32, tag="st")
                            nc.tensor.matmul(
                                bc_ps[:, 0:512],
                                ones_r[0:1, :], recip[0:1, :],
                                start=True, stop=True,
                            )
                            bc_sb = workS.tile([128, 512], BF16, tag="bc")
                            nc.vector.tensor_copy(
                                out=bc_sb[orows[0]:orows[1], :],
                                in_=bc_ps[orows[0]:orows[1], 0:512],
                            )
                            nc.vector.tensor_tensor(
                                out=otn[orows[0]:orows[1], c, q0 + qq * 512:q0 + (qq + 1) * 512],
                                in0=ot[0:HD, :],
                                in1=bc_sb[orows[0]:orows[1], :], op=MULT,
                            )

                        flush_norm(1)
                        pending_norm.append(norm)

                qproj_fill = [(mc, lc) for mc in range(MC) for lc in range(2, 4)]
                fill_xt = {}
                for h in range(HC):
                    emit_attention_head(h, 0)
                    if h < len(qproj_fill):
                        mc, lc = qproj_fill[h]
                        if lc not in fill_xt:
                            xt = xtpool.tile([128, DCH, 512], BF16)
                            nc.sync.dma_start(
                                out=xt,
                                in_=xT_d.rearrange("(c p) l -> p c l", p=128)[:, :, lc * 512:(lc + 1) * 512],
                            )
                            fill_xt[lc] = xt
                        xt = fill_xt[lc]
                        q_ps = psA.tile([128, 512], F32, tag="pa")
                        for k in range(DCH):
                            nc.tensor.matmul(
                                q_ps[:, :],
                                wqk_r[:, k, mc * 128:(mc + 1) * 128],
                                xt[:, k, :],
                                start=(k == 0), stop=(k == DCH - 1),
                            )
                        nc.vector.tensor_scalar_add(
                            out=qT[:, mc, lc * 512:(lc + 1) * 512],
                            in0=q_ps, scalar1=bqk_t[:, mc:mc + 1],
                        )

            # ------------- attention hemi 1 (+ out proj hemi 0) ------------
            with tc.tile_pool(name="psD", bufs=2, space="PSUM") as psD:
                def emit_outproj(qt):
                    y_sb = workS.tile([128, DOUT], BF16, tag="y")
                    for dc in range(DC):
                        y_ps = psD.tile([128, 512], F32, tag="y")
                        for pr in range(MC):
                            nc.tensor.matmul(
                                y_ps[:, :],
                                otn[:, pr, qt * 128:(qt + 1) * 128],
                                wo_r[:, pr, dc * 512:(dc + 1) * 512],
                                start=(pr == 0), stop=False,
                            )
                        nc.tensor.matmul(
                            y_ps[:, :], ones_r[0:1, :], bo_r[0:1, dc * 512:(dc + 1) * 512],
                            start=False, stop=True,
                        )
                        nc.vector.tensor_copy(
                            out=y_sb[:, dc * 512:(dc + 1) * 512], in_=y_ps,
                        )
                    nc.sync.dma_start(out=y_d[qt * 128:(qt + 1) * 128, :], in_=y_sb)

                # interleave: out-proj of hemi-0 query tiles (qt 0..7) rides
                # inside hemi-1 attention; hemi-1 tiles (qt 8..15) can only
                # start after the last head's attention — emitted as the tail.
                for h in range(HC):
                    emit_attention_head(h, 1)
                    emit_outproj(h)
                flush_norm(-1)
                for qt in range(L // 256, L // 128):
                    emit_outproj(qt)

    # split multi-waits (walrus allows 1 sync wait per instruction reliably)
    if waitsplit:
        _split_excess_waits(nc)
    return nc


def _plan(mask, L, D, H):
    """Shared cfg incl. padded valid-key count (multiple of 128)."""
    valid = (~np.asarray(mask, bool)).sum(axis=1)
    lv = int(valid.max())
    lv_pad = max(128, min(L, ((lv + 127) // 128) * 128))
    return {"L": L, "D": D, "HC": H // 2, "HD": D // H, "Lv": lv_pad}


def _prep_inputs(x, mask, W_qkv, b_qkv, W_out, b_out, cfg):
    """Build the 8 per-core input maps (host-side shuffles)."""
    import ml_dtypes

    BF = ml_dtypes.bfloat16
    L, D, HC, HD, Lv = cfg["L"], cfg["D"], cfg["HC"], cfg["HD"], cfg["Lv"]
    DV = HC * HD
    MC = DV // 128
    N = x.shape[0]
    scale = 1.0 / np.sqrt(HD)
    Wt = np.ascontiguousarray(W_qkv.T).astype(np.float32)    # [D, 3D]
    WoT = np.ascontiguousarray(W_out.T).astype(np.float32)   # [D, D]
    DCH = D // 128
    KC = Lv // 128

    per_hg = []
    for hg in range(2):
        qs, ks, vs = hg * DV, D + hg * DV, 2 * D + hg * DV
        wqk = np.concatenate(
            [Wt[:, qs:qs + DV] * scale, Wt[:, ks:ks + DV]], axis=1
        )  # [D, 2DV]
        wqk = wqk.reshape(DCH, 128, 2 * DV)
        wqk = np.ascontiguousarray(wqk.transpose(1, 0, 2)).astype(BF)
        wv = Wt[:, vs:vs + DV].reshape(DCH, 128, DV)
        wv = np.ascontiguousarray(wv.transpose(1, 0, 2)).astype(BF)
        bqk = np.concatenate(
            [b_qkv[qs:qs + DV] * scale, b_qkv[ks:ks + DV]]
        ).reshape(2 * MC, 128)
        bqk = np.ascontiguousarray(bqk.T).astype(np.float32)  # [128, 2MC]
        bv = np.ascontiguousarray(b_qkv[vs:vs + DV][None, :]).astype(np.float32)
        # wo: [128, MC, D] — head-pair packed rows (pair pr = heads 2pr,2pr+1)
        wo_heads = WoT[hg * DV:(hg + 1) * DV, :].reshape(HC, HD, D)
        wo = np.ascontiguousarray(
            wo_heads.reshape(MC, 2 * HD, D).transpose(1, 0, 2)
        ).astype(BF)
        per_hg.append(dict(wqk=wqk, wv=wv, bqk=bqk, bv=bv, wo=wo))

    # b_out only on hg=0 cores; partials are summed on host (avoid 2x bias)
    bo_full = np.ascontiguousarray(b_out[None, :]).astype(np.float32)
    bo_zero = np.zeros_like(bo_full)
    xTs, xkTs, mbs = [], [], []
    for n in range(N):
        xTs.append(np.ascontiguousarray(x[n].T).astype(BF))
        kidx = np.nonzero(~np.asarray(mask[n], bool))[0]
        xk = np.zeros((Lv, D), np.float32)
        xk[:len(kidx)] = x[n][kidx]
        xkTs.append(np.ascontiguousarray(xk.T).astype(BF))
        mb = np.full(Lv, -1e9, np.float32)
        mb[:len(kidx)] = 0.0
        mbs.append(np.ascontiguousarray(mb.reshape(KC, 128).T))

    in_maps = []
    for c in range(2 * N):
        n, hg = c // 2, c % 2
        d = dict(per_hg[hg])
        d.update(xT=xTs[n], xkT=xkTs[n], mb=mbs[n],
                 bo=(bo_full if hg == 0 else bo_zero))
        in_maps.append(d)
    return in_maps


def kernel(x, mask, W_qkv, b_qkv, W_out, b_out):
    from concourse.bass_utils import run_bass_kernel_spmd

    x = np.asarray(x, dtype=np.float32)
    mask = np.asarray(mask)
    N, L, D = x.shape
    H = 16
    cfg = _plan(mask, L, D, H)

    key = (L, D, H, cfg["Lv"])
    if key not in _KERNEL_CACHE:
        _KERNEL_CACHE[key] = _build(cfg)
    nc = _KERNEL_CACHE[key]

    in_maps = _prep_inputs(
        x, mask,
        np.asarray(W_qkv, np.float32), np.asarray(b_qkv, np.float32),
        np.asarray(W_out, np.float32), np.asarray(b_out, np.float32), cfg,
    )
    res = run_bass_kernel_spmd(nc, in_maps, list(range(2 * N)))
    out = np.empty((N, L, D), np.float32)
    for n in range(N):
        out[n] = (np.asarray(res.results[2 * n]["y"]).astype(np.float32)
                  + np.asarray(res.results[2 * n + 1]["y"]).astype(np.float32))
    return out


# revision 29
# speedup vs baseline: 1.2447x; 1.2447x over previous
"""MiniMHSA Trainium2 kernel: 8 NeuronCores, shard = (batch n, head-group).

Reference computes, per batch n:
  qkv = x @ W_qkv.T + b_qkv ; split into q,k,v heads (H=16, HD=64)
  scores = (q @ k.T) / sqrt(HD), masked keys -> -1e9, softmax, @ v
  out = attn_out @ W_out.T + b_out

Core c handles n = c//2 and head-group hg = c%2 (8 heads each).

Key ideas vs the naive version:
  * Mask compaction: masked keys get exp(-1e9)=0 exactly, so the host
    gathers only the valid keys (~half) into xkT[D, Lv]; k/v projection,
    scores, exp and AV all shrink by ~2x. Pad keys carry bias -1e9.
  * bf16 operands everywhere on the PE (same 1 cycle/row as f32r, half
    the SBUF/DMA), fp32 PSUM accumulation throughout.
  * Single pass over all 8 heads; attention output otn kept in SBUF as
    head PAIRS on 128 partitions (even head rows 0:64, odd rows 64:128)
    so the out-projection contracts 128-deep. Odd heads put the softmax
    denominator at PSUM row 63 by placing the ones column FIRST in the
    stationary v tile.
  * Software-pipelined emission: q-proj of hemi 1 rides inside hemi-0
    attention; out-proj of hemi 0 rides inside hemi-1 attention. The
    softmax exp (scalar engine, the true bottleneck) never waits.
  * y is DMAed straight from PSUM (no DVE copy).
"""
import sys

sys.path.insert(0, '/opt/trn_rl_repo')


import numpy as np

_KERNEL_CACHE = {}


def _split_excess_waits(nc):
    """Walrus codegen reliably accepts only ONE sync wait per instruction
    (Matmult hard-fails at 2, Drain at 5). Tile's scheduler can attach more.
    Move excess waits onto preceding same-engine NOPs — semantically identical
    since engine queues execute in order."""
    from concourse import mybir

    for f in nc.m.functions:
        for blk in f.blocks:
            il = blk.instructions
            i = 0
            while i < len(il):
                inst = il[i]
                si = inst.sync_info
                waits = list(si.on_wait) if si is not None and si.on_wait else []
                if len(waits) > 1:
                    keep = waits[-1:]
                    excess = waits[:-1]
                    pos = i
                    for j, wcond in enumerate(excess):
                        nop = mybir.InstNoOp(name=f"{inst.name}-ws{j}", ins=[], outs=[])
                        nop.engine = inst.engine
                        nop.sync_info = mybir.SyncInfo(on_wait=[wcond], on_update=[])
                        il.insert(pos, nop)
                        pos += 1
                        i += 1
                    inst.sync_info = mybir.SyncInfo(
                        on_wait=keep,
                        on_update=list(si.on_update) if si.on_update else [],
                    )
                i += 1


def _build(cfg, waitsplit=True):
    import concourse.bass as bass
    import concourse.tile as tile
    from concourse import mybir

    F32 = mybir.dt.float32
    F32R = mybir.dt.float32r
    BF16 = mybir.dt.bfloat16
    FP8 = mybir.dt.float8e4
    DR = mybir.MatmulPerfMode.DoubleRow
    AF = mybir.ActivationFunctionType
    MULT = mybir.AluOpType.mult

    L, D, HC, HD = cfg["L"], cfg["D"], cfg["HC"], cfg["HD"]
    Lv = cfg["Lv"]            # padded valid-key count (multiple of 128)
    DCH = D // 128            # contraction chunks for projections
    DV = HC * HD              # qkv width per core (512)
    MC = DV // 128            # m-chunks for q (and for k) = head pairs = 4
    KC = Lv // 128            # attention key chunks
    QH = L // 1024            # attention q hemis (1024 wide)
    DOUT = D
    DC = DOUT // 512

    nc = bass.Bass()
    xT_d = nc.dram_tensor("xT", [D, L], BF16, kind="ExternalInput")
    xkT_d = nc.dram_tensor("xkT", [D, Lv], BF16, kind="ExternalInput")
    wqk_d = nc.dram_tensor("wqk", [128, DCH, 2 * DV], BF16, kind="ExternalInput")
    wv_d = nc.dram_tensor("wv", [128, DCH, DV], BF16, kind="ExternalInput")
    bqk_d = nc.dram_tensor("bqk", [128, 2 * MC], F32, kind="ExternalInput")
    bv_d = nc.dram_tensor("bv", [1, DV], F32, kind="ExternalInput")
    mb_d = nc.dram_tensor("mb", [128, KC], F32, kind="ExternalInput")
    wo_d = nc.dram_tensor("wo", [128, MC, DOUT], BF16, kind="ExternalInput")
    bo_d = nc.dram_tensor("bo", [1, DOUT], F32, kind="ExternalInput")
    y_d = nc.dram_tensor("y", [L, DOUT], BF16, kind="ExternalOutput")

    # k/v projections consume compacted keys in chunks of <=512 columns
    kv_chunks = []
    off = 0
    while off < Lv:
        w = min(512, Lv - off)
        kv_chunks.append((off, w))
        off += w

    with tile.TileContext(nc) as tc, \
         nc.allow_low_precision(reason="bf16 matmuls intended"):
        with tc.tile_pool(name="const", bufs=1) as const, \
             tc.tile_pool(name="wpool", bufs=1) as wpool, \
             tc.tile_pool(name="big", bufs=1) as big, \
             tc.tile_pool(name="xkpool", bufs=2) as xkpool, \
             tc.tile_pool(name="xtpool", bufs=2) as xtpool, \
             tc.tile_pool(name="workP", bufs=12) as workP, \
             tc.tile_pool(name="workS", bufs=2) as workS, \
             tc.tile_pool(name="psB", bufs=2, space="PSUM") as psB, \
             tc.tile_pool(name="psC", bufs=2, space="PSUM") as psC:

            # ---- weight/const DMAs (pool queue), x DMAs (sync queue) ----
            wqk_r = wpool.tile([128, DCH, 2 * DV], BF16)
            # k half first: kv projection starts as soon as possible
            nc.gpsimd.dma_start(out=wqk_r[:, :, DV:2 * DV], in_=wqk_d[:, :, DV:2 * DV])
            wv_r = wpool.tile([128, DCH, DV], BF16)
            nc.gpsimd.dma_start(out=wv_r, in_=wv_d[:, :, :])
            nc.gpsimd.dma_start(out=wqk_r[:, :, 0:DV], in_=wqk_d[:, :, 0:DV])
            bqk_t = const.tile([128, 2 * MC], F32)
            nc.gpsimd.dma_start(out=bqk_t, in_=bqk_d[:, :])
            mb_t = const.tile([128, KC], F32)
            nc.gpsimd.dma_start(out=mb_t, in_=mb_d[:, :])
            bv_r = const.tile([1, DV], F32R)
            nc.gpsimd.dma_start(out=bv_r, in_=bv_d[:, :])
            wo_r = wpool.tile([128, MC, DOUT], BF16)
            nc.gpsimd.dma_start(out=wo_r, in_=wo_d[:, :, :])
            bo_r = const.tile([1, DOUT], F32R)
            nc.gpsimd.dma_start(out=bo_r, in_=bo_d[:, :])

            ones_f = const.tile([128, 1], F32)
            nc.vector.memset(ones_f, 1.0)
            ones_r = const.tile([1, 128], F32R)
            nc.vector.tensor_copy(out=ones_r, in_=ones_f[0:1, 0:1].broadcast_to([1, 128]))

            # ---- persistent SBUF state ----
            qT = big.tile([128, MC, L], BF16, tag="qT")
            kT = big.tile([128, MC, Lv], BF16, tag="kT")
            # vp cols: 0..63 = v, 64 = ones (softmax denominator column in
            # the transposed AV below). bf16: fp8 quantization noise (~3%)
            # would exceed the 2e-2 error budget.
            vp = big.tile([128, KC, HC, HD + 1], BF16, tag="vp")
            nc.vector.memset(vp[:, :, :, HD:HD + 1], 1.0)
            otn = big.tile([128, MC, L], BF16, tag="otn")

            # ---------------- k/v projection (compacted keys) --------------
            with tc.tile_pool(name="psA", bufs=2, space="PSUM") as psA:
                for (off, w) in kv_chunks:
                    xkt = xkpool.tile([128, DCH, 512], BF16)
                    nc.sync.dma_start(
                        out=xkt[:, :, 0:w],
                        in_=xkT_d.rearrange("(c p) l -> p c l", p=128)[:, :, off:off + w],
                    )
                    for mc in range(MC):
                        k_ps = psA.tile([128, 512], F32, tag="pa")
                        for k in range(DCH):
                            nc.tensor.matmul(
                                k_ps[:, 0:w],
                                wqk_r[:, k, DV + mc * 128:DV + (mc + 1) * 128],
                                xkt[:, k, 0:w],
                                start=(k == 0), stop=(k == DCH - 1),
                            )
                        nc.vector.tensor_scalar_add(
                            out=kT[:, mc, off:off + w],
                            in0=k_ps[:, 0:w], scalar1=bqk_t[:, MC + mc:MC + mc + 1],
                        )
                    for sub in range(w // 128):
                        kcg = (off + sub * 128) // 128
                        v_ps = psA.tile([128, DV], F32, tag="pa")
                        for k in range(DCH):
                            nc.tensor.matmul(
                                v_ps[:, :],
                                xkt[:, k, sub * 128:(sub + 1) * 128],
                                wv_r[:, k, :],
                                start=(k == 0), stop=False,
                            )
                        nc.tensor.matmul(
                            v_ps[:, :], ones_r[0:1, :], bv_r[0:1, :],
                            start=False, stop=True,
                        )
                        nc.vector.tensor_copy(
                            out=vp[:, kcg, :, 0:HD],
                            in_=v_ps.rearrange("p (h d) -> p h d", h=HC),
                        )

                # ------------- q projection, hemi 0 ------------------------
                def emit_qproj(lc):
                    xt = xtpool.tile([128, DCH, 512], BF16)
                    nc.sync.dma_start(
                        out=xt,
                        in_=xT_d.rearrange("(c p) l -> p c l", p=128)[:, :, lc * 512:(lc + 1) * 512],
                    )
                    for mc in range(MC):
                        q_ps = psA.tile([128, 512], F32, tag="pa")
                        for k in range(DCH):
                            nc.tensor.matmul(
                                q_ps[:, :],
                                wqk_r[:, k, mc * 128:(mc + 1) * 128],
                                xt[:, k, :],
                                start=(k == 0), stop=(k == DCH - 1),
                            )
                        nc.vector.tensor_scalar_add(
                            out=qT[:, mc, lc * 512:(lc + 1) * 512],
                            in0=q_ps, scalar1=bqk_t[:, mc:mc + 1],
                        )

                for lc in range(2):
                    emit_qproj(lc)

                # ------------- attention hemi 0 (+ q proj hemi 1) ----------
                # Transposed AV: pT is the STATIONARY operand [128k, 128q],
                # v the moving one (65 cols) — AV matmul cost drops 8x per
                # column count, and the softmax denominator lands in a
                # COLUMN, so normalize is a per-partition reciprocal +
                # tensor_scalar — no PE broadcast, no PSUM copies. Head
                # pairs share one [128q, 128] normalized tile (even head
                # cols 0:64, odd cols 64:128) whose DMA-transpose is
                # exactly the paired otn layout.
                on_pairs = {}

                def emit_attention_head(h, qh):
                    c, par = h // 2, h % 2
                    q0 = qh * 1024
                    pts = []
                    for kc in range(KC):
                        st = psB.tile([128, 1024], F32, tag="st")
                        for s in range(2):
                            nc.tensor.matmul(
                                st[:, s * 512:(s + 1) * 512],
                                kT[64 * par:64 * par + 64, c, kc * 128:(kc + 1) * 128],
                                qT[64 * par:64 * par + 64, c, q0 + s * 512:q0 + (s + 1) * 512],
                                start=True, stop=True,
                            )
                        pt = workP.tile([128, 1024], BF16, tag="pT")
                        nc.scalar.activation(
                            out=pt, in_=st, func=AF.Exp,
                            bias=mb_t[:, kc:kc + 1], scale=1.0,
                        )
                        pts.append(pt)
                    if par == 0:
                        on_pairs[(c, qh)] = [None] * 8
                    for qc in range(8):
                        ot2 = psC.tile([128, 512], F32, tag="ot")
                        for kc in range(KC):
                            nc.tensor.matmul(
                                ot2[:, 0:HD + 1],
                                pts[kc][:, qc * 128:(qc + 1) * 128],
                                vp[:, kc, h, :],
                                start=(kc == 0), stop=(kc == KC - 1),
                            )
                        recip2 = workS.tile([128, 1], F32, tag="rc")
                        nc.vector.reciprocal(out=recip2, in_=ot2[:, HD:HD + 1])
                        if par == 0:
                            onp = workS.tile([128, 128], BF16, tag="on", bufs=16)
                            on_pairs[(c, qh)][qc] = onp
                            tgt = onp[:, 0:HD]
                        else:
                            onp = on_pairs[(c, qh)][qc]
                            tgt = onp[:, HD:128]
                        nc.vector.tensor_scalar_mul(
                            out=tgt, in0=ot2[:, 0:HD], scalar1=recip2,
                        )
                        if par == 1:
                            nc.sync.dma_start(
                                out=otn[:, c, q0 + qc * 128:q0 + (qc + 1) * 128],
                                in_=onp, transpose=True,
                            )

                qproj_fill = [(mc, lc) for mc in range(MC) for lc in range(2, 4)]
                fill_xt = {}
                for h in range(HC):
                    emit_attention_head(h, 0)
                    if h < len(qproj_fill):
                        mc, lc = qproj_fill[h]
                        if lc not in fill_xt:
                            xt = xtpool.tile([128, DCH, 512], BF16)
                            nc.sync.dma_start(
                                out=xt,
                                in_=xT_d.rearrange("(c p) l -> p c l", p=128)[:, :, lc * 512:(lc + 1) * 512],
                            )
                            fill_xt[lc] = xt
                        xt = fill_xt[lc]
                        q_ps = psA.tile([128, 512], F32, tag="pa")
                        for k in range(DCH):
                            nc.tensor.matmul(
                                q_ps[:, :],
                                wqk_r[:, k, mc * 128:(mc + 1) * 128],
                                xt[:, k, :],
                                start=(k == 0), stop=(k == DCH - 1),
                            )
                        nc.vector.tensor_scalar_add(
                            out=qT[:, mc, lc * 512:(lc + 1) * 512],
                            in0=q_ps, scalar1=bqk_t[:, mc:mc + 1],
                        )

            # ------------- attention hemi 1 (+ out proj hemi 0) ------------
            with tc.tile_pool(name="psD", bufs=2, space="PSUM") as psD:
                def emit_outproj(qt):
                    y_sb = workS.tile([128, DOUT], BF16, tag="y")
                    for dc in range(DC):
                        y_ps = psD.tile([128, 512], F32, tag="y")
                        for pr in range(MC):
                            nc.tensor.matmul(
                                y_ps[:, :],
                                otn[:, pr, qt * 128:(qt + 1) * 128],
                                wo_r[:, pr, dc * 512:(dc + 1) * 512],
                                start=(pr == 0), stop=False,
                            )
                        nc.tensor.matmul(
                            y_ps[:, :], ones_r[0:1, :], bo_r[0:1, dc * 512:(dc + 1) * 512],
                            start=False, stop=True,
                        )
                        nc.vector.tensor_copy(
                            out=y_sb[:, dc * 512:(dc + 1) * 512], in_=y_ps,
                        )
                    nc.sync.dma_start(out=y_d[qt * 128:(qt + 1) * 128, :], in_=y_sb)

                # interleave: out-proj of hemi-0 query tiles (qt 0..7) rides
                # inside hemi-1 attention; hemi-1 tiles (qt 8..15) can only
                # start after the last head's attention — emitted as the tail.
                for h in range(HC):
                    emit_attention_head(h, 1)
                    emit_outproj(h)
                for qt in range(L // 256, L // 128):
                    emit_outproj(qt)

    # split multi-waits (walrus allows 1 sync wait per instruction reliably)
    if waitsplit:
        _split_excess_waits(nc)
    return nc


def _plan(mask, L, D, H):
    """Shared cfg incl. padded valid-key count (multiple of 128)."""
    valid = (~np.asarray(mask, bool)).sum(axis=1)
    lv = int(valid.max())
    lv_pad = max(128, min(L, ((lv + 127) // 128) * 128))
    return {"L": L, "D": D, "HC": H // 2, "HD": D // H, "Lv": lv_pad}


def _prep_inputs(x, mask, W_qkv, b_qkv, W_out, b_out, cfg):
    """Build the 8 per-core input maps (host-side shuffles)."""
    import ml_dtypes

    BF = ml_dtypes.bfloat16
    L, D, HC, HD, Lv = cfg["L"], cfg["D"], cfg["HC"], cfg["HD"], cfg["Lv"]
    DV = HC * HD
    MC = DV // 128
    N = x.shape[0]
    scale = 1.0 / np.sqrt(HD)
    Wt = np.ascontiguousarray(W_qkv.T).astype(np.float32)    # [D, 3D]
    WoT = np.ascontiguousarray(W_out.T).astype(np.float32)   # [D, D]
    DCH = D // 128
    KC = Lv // 128

    per_hg = []
    for hg in range(2):
        qs, ks, vs = hg * DV, D + hg * DV, 2 * D + hg * DV
        wqk = np.concatenate(
            [Wt[:, qs:qs + DV] * scale, Wt[:, ks:ks + DV]], axis=1
        )  # [D, 2DV]
        wqk = wqk.reshape(DCH, 128, 2 * DV)
        wqk = np.ascontiguousarray(wqk.transpose(1, 0, 2)).astype(BF)
        wv = Wt[:, vs:vs + DV].reshape(DCH, 128, DV)
        wv = np.ascontiguousarray(wv.transpose(1, 0, 2)).astype(BF)
        bqk = np.concatenate(
            [b_qkv[qs:qs + DV] * scale, b_qkv[ks:ks + DV]]
        ).reshape(2 * MC, 128)
        bqk = np.ascontiguousarray(bqk.T).astype(np.float32)  # [128, 2MC]
        bv = np.ascontiguousarray(b_qkv[vs:vs + DV][None, :]).astype(np.float32)
        # wo: [128, MC, D] — head-pair packed rows (pair pr = heads 2pr,2pr+1)
        wo_heads = WoT[hg * DV:(hg + 1) * DV, :].reshape(HC, HD, D)
        wo = np.ascontiguousarray(
            wo_heads.reshape(MC, 2 * HD, D).transpose(1, 0, 2)
        ).astype(BF)
        per_hg.append(dict(wqk=wqk, wv=wv, bqk=bqk, bv=bv, wo=wo))

    # b_out only on hg=0 cores; partials are summed on host (avoid 2x bias)
    bo_full = np.ascontiguousarray(b_out[None, :]).astype(np.float32)
    bo_zero = np.zeros_like(bo_full)
    xTs, xkTs, mbs = [], [], []
    for n in range(N):
        xTs.append(np.ascontiguousarray(x[n].T).astype(BF))
        kidx = np.nonzero(~np.asarray(mask[n], bool))[0]
        xk = np.zeros((Lv, D), np.float32)
        xk[:len(kidx)] = x[n][kidx]
        xkTs.append(np.ascontiguousarray(xk.T).astype(BF))
        mb = np.full(Lv, -1e9, np.float32)
        mb[:len(kidx)] = 0.0
        mbs.append(np.ascontiguousarray(mb.reshape(KC, 128).T))

    in_maps = []
    for c in range(2 * N):
        n, hg = c // 2, c % 2
        d = dict(per_hg[hg])
        d.update(xT=xTs[n], xkT=xkTs[n], mb=mbs[n],
                 bo=(bo_full if hg == 0 else bo_zero))
        in_maps.append(d)
    return in_maps


def kernel(x, mask, W_qkv, b_qkv, W_out, b_out):
    from concourse.bass_utils import run_bass_kernel_spmd

    x = np.asarray(x, dtype=np.float32)
    mask = np.asarray(mask)
    N, L, D = x.shape
    H = 16
    cfg = _plan(mask, L, D, H)

    key = (L, D, H, cfg["Lv"])
    if key not in _KERNEL_CACHE:
        _KERNEL_CACHE[key] = _build(cfg)
    nc = _KERNEL_CACHE[key]

    in_maps = _prep_inputs(
        x, mask,
        np.asarray(W_qkv, np.float32), np.asarray(b_qkv, np.float32),
        np.asarray(W_out, np.float32), np.asarray(b_out, np.float32), cfg,
    )
    res = run_bass_kernel_spmd(nc, in_maps, list(range(2 * N)))
    out = np.empty((N, L, D), np.float32)
    for n in range(N):
        out[n] = (np.asarray(res.results[2 * n]["y"]).astype(np.float32)
                  + np.asarray(res.results[2 * n + 1]["y"]).astype(np.float32))
    return out


# revision 31
# speedup vs baseline: 1.2937x; 1.0393x over previous
"""MiniMHSA Trainium2 kernel: 8 NeuronCores, shard = (batch n, head-group).

Reference computes, per batch n:
  qkv = x @ W_qkv.T + b_qkv ; split into q,k,v heads (H=16, HD=64)
  scores = (q @ k.T) / sqrt(HD), masked keys -> -1e9, softmax, @ v
  out = attn_out @ W_out.T + b_out

Core c handles n = c//2 and head-group hg = c%2 (8 heads each).

Key ideas vs the naive version:
  * Mask compaction: masked keys get exp(-1e9)=0 exactly, so the host
    gathers only the valid keys (~half) into xkT[D, Lv]; k/v projection,
    scores, exp and AV all shrink by ~2x. Pad keys carry bias -1e9.
  * bf16 operands everywhere on the PE (same 1 cycle/row as f32r, half
    the SBUF/DMA), fp32 PSUM accumulation throughout.
  * Single pass over all 8 heads; attention output otn kept in SBUF as
    head PAIRS on 128 partitions (even head rows 0:64, odd rows 64:128)
    so the out-projection contracts 128-deep. Odd heads put the softmax
    denominator at PSUM row 63 by placing the ones column FIRST in the
    stationary v tile.
  * Software-pipelined emission: q-proj of hemi 1 rides inside hemi-0
    attention; out-proj of hemi 0 rides inside hemi-1 attention. The
    softmax exp (scalar engine, the true bottleneck) never waits.
  * y is DMAed straight from PSUM (no DVE copy).
"""
import sys

sys.path.insert(0, '/opt/trn_rl_repo')


import numpy as np

_KERNEL_CACHE = {}


def _split_excess_waits(nc):
    """Walrus codegen reliably accepts only ONE sync wait per instruction
    (Matmult hard-fails at 2, Drain at 5). Tile's scheduler can attach more.
    Move excess waits onto preceding same-engine NOPs — semantically identical
    since engine queues execute in order."""
    from concourse import mybir

    for f in nc.m.functions:
        for blk in f.blocks:
            il = blk.instructions
            i = 0
            while i < len(il):
                inst = il[i]
                si = inst.sync_info
                waits = list(si.on_wait) if si is not None and si.on_wait else []
                if len(waits) > 1:
                    keep = waits[-1:]
                    excess = waits[:-1]
                    pos = i
                    for j, wcond in enumerate(excess):
                        nop = mybir.InstNoOp(name=f"{inst.name}-ws{j}", ins=[], outs=[])
                        nop.engine = inst.engine
                        nop.sync_info = mybir.SyncInfo(on_wait=[wcond], on_update=[])
                        il.insert(pos, nop)
                        pos += 1
                        i += 1
                    inst.sync_info = mybir.SyncInfo(
                        on_wait=keep,
                        on_update=list(si.on_update) if si.on_update else [],
                    )
                i += 1


def _build(cfg, waitsplit=True):
    import concourse.bass as bass
    import concourse.tile as tile
    from concourse import mybir

    F32 = mybir.dt.float32
    F32R = mybir.dt.float32r
    BF16 = mybir.dt.bfloat16
    FP8 = mybir.dt.float8e4
    DR = mybir.MatmulPerfMode.DoubleRow
    AF = mybir.ActivationFunctionType
    MULT = mybir.AluOpType.mult

    L, D, HC, HD = cfg["L"], cfg["D"], cfg["HC"], cfg["HD"]
    Lv = cfg["Lv"]            # padded valid-key count (multiple of 128)
    DCH = D // 128            # contraction chunks for projections
    DV = HC * HD              # qkv width per core (512)
    MC = DV // 128            # m-chunks for q (and for k) = head pairs = 4
    KC = Lv // 128            # attention key chunks
    QH = L // 1024            # attention q hemis (1024 wide)
    DOUT = D
    DC = DOUT // 512

    nc = bass.Bass()
    xT_d = nc.dram_tensor("xT", [D, L], BF16, kind="ExternalInput")
    xkT_d = nc.dram_tensor("xkT", [D, Lv], BF16, kind="ExternalInput")
    wqk_d = nc.dram_tensor("wqk", [128, DCH, 2 * DV], BF16, kind="ExternalInput")
    wv_d = nc.dram_tensor("wv", [128, DCH, DV], BF16, kind="ExternalInput")
    bqk_d = nc.dram_tensor("bqk", [128, 2 * MC], F32, kind="ExternalInput")
    bv_d = nc.dram_tensor("bv", [1, DV], F32, kind="ExternalInput")
    mb_d = nc.dram_tensor("mb", [128, KC], F32, kind="ExternalInput")
    wo_d = nc.dram_tensor("wo", [128, MC, DOUT], BF16, kind="ExternalInput")
    bo_d = nc.dram_tensor("bo", [1, DOUT], F32, kind="ExternalInput")
    y_d = nc.dram_tensor("y", [L, DOUT], BF16, kind="ExternalOutput")

    # k/v projections consume compacted keys in chunks of <=512 columns
    kv_chunks = []
    off = 0
    while off < Lv:
        w = min(512, Lv - off)
        kv_chunks.append((off, w))
        off += w

    with tile.TileContext(nc) as tc, \
         nc.allow_low_precision(reason="bf16 matmuls intended"):
        with tc.tile_pool(name="const", bufs=1) as const, \
             tc.tile_pool(name="wpool", bufs=1) as wpool, \
             tc.tile_pool(name="big", bufs=1) as big, \
             tc.tile_pool(name="xkpool", bufs=3) as xkpool, \
             tc.tile_pool(name="xtpool", bufs=4) as xtpool, \
             tc.tile_pool(name="workP", bufs=12) as workP, \
             tc.tile_pool(name="workS", bufs=2) as workS, \
             tc.tile_pool(name="psB", bufs=2, space="PSUM") as psB, \
             tc.tile_pool(name="psC", bufs=2, space="PSUM") as psC:

            # ---- weight/const DMAs (pool queue), x DMAs (sync queue) ----
            wqk_r = wpool.tile([128, DCH, 2 * DV], BF16)
            # k half first: kv projection starts as soon as possible
            nc.gpsimd.dma_start(out=wqk_r[:, :, DV:2 * DV], in_=wqk_d[:, :, DV:2 * DV])
            wv_r = wpool.tile([128, DCH, DV], BF16)
            nc.gpsimd.dma_start(out=wv_r, in_=wv_d[:, :, :])
            nc.gpsimd.dma_start(out=wqk_r[:, :, 0:DV], in_=wqk_d[:, :, 0:DV])
            bqk_t = const.tile([128, 2 * MC], F32)
            nc.gpsimd.dma_start(out=bqk_t, in_=bqk_d[:, :])
            mb_t = const.tile([128, KC], F32)
            nc.gpsimd.dma_start(out=mb_t, in_=mb_d[:, :])
            bv_r = const.tile([1, DV], F32R)
            nc.gpsimd.dma_start(out=bv_r, in_=bv_d[:, :])
            wo_r = wpool.tile([128, MC, DOUT], BF16)
            nc.gpsimd.dma_start(out=wo_r, in_=wo_d[:, :, :])
            bo_r = const.tile([1, DOUT], F32R)
            nc.gpsimd.dma_start(out=bo_r, in_=bo_d[:, :])

            ones_f = const.tile([128, 1], F32)
            nc.vector.memset(ones_f, 1.0)
            ones_r = const.tile([1, 128], F32R)
            nc.vector.tensor_copy(out=ones_r, in_=ones_f[0:1, 0:1].broadcast_to([1, 128]))

            # ---- persistent SBUF state ----
            qT = big.tile([128, MC, L], BF16, tag="qT")
            kT = big.tile([128, MC, Lv], BF16, tag="kT")
            # vp cols: 0..63 = v, 64 = ones (softmax denominator column in
            # the transposed AV below). bf16: fp8 quantization noise (~3%)
            # would exceed the 2e-2 error budget.
            vp = big.tile([128, KC, HC, HD + 1], BF16, tag="vp")
            nc.vector.memset(vp[:, :, :, HD:HD + 1], 1.0)
            otn = big.tile([128, MC, L], BF16, tag="otn")

            # ---------------- projections + attention, one PE stream ----
            # Only k-proj and the first q pair run BEFORE attention; v-proj
            # and the remaining q-proj groups ride as ~1.8us "fills" inside
            # the exp-paced attention window (PE has ~3.5us slack per
            # head-hemi vs the scalar engine). AV runs one head LATE so it
            # never waits on exp. Forced drains guarantee a fill is emitted
            # before any instruction that consumes its output.
            with tc.tile_pool(name="psW", bufs=2, space="PSUM") as psW:
                xkts = []
                for (off, w) in kv_chunks:
                    xkt = xkpool.tile([128, DCH, 512], BF16)
                    nc.sync.dma_start(
                        out=xkt[:, :, 0:w],
                        in_=xkT_d.rearrange("(c p) l -> p c l", p=128)[:, :, off:off + w],
                    )
                    xkts.append((xkt, off, w))
                for (xkt, off, w) in xkts:
                    for mc in range(MC):
                        k_ps = psW.tile([128, 512], F32, tag="pa")
                        for k in range(DCH):
                            nc.tensor.matmul(
                                k_ps[:, 0:w],
                                wqk_r[:, k, DV + mc * 128:DV + (mc + 1) * 128],
                                xkt[:, k, 0:w],
                                start=(k == 0), stop=(k == DCH - 1),
                            )
                        nc.vector.tensor_scalar_add(
                            out=kT[:, mc, off:off + w],
                            in0=k_ps[:, 0:w], scalar1=bqk_t[:, MC + mc:MC + mc + 1],
                        )

                fill_xt = {}

                def q_group(mc, lc):
                    if lc not in fill_xt:
                        xt = xtpool.tile([128, DCH, 512], BF16)
                        nc.sync.dma_start(
                            out=xt,
                            in_=xT_d.rearrange("(c p) l -> p c l", p=128)[:, :, lc * 512:(lc + 1) * 512],
                        )
                        fill_xt[lc] = xt
                    xt = fill_xt[lc]
                    q_ps = psW.tile([128, 512], F32, tag="pa")
                    for k in range(DCH):
                        nc.tensor.matmul(
                            q_ps[:, :],
                            wqk_r[:, k, mc * 128:(mc + 1) * 128],
                            xt[:, k, :],
                            start=(k == 0), stop=(k == DCH - 1),
                        )
                    nc.vector.tensor_scalar_add(
                        out=qT[:, mc, lc * 512:(lc + 1) * 512],
                        in0=q_ps, scalar1=bqk_t[:, mc:mc + 1],
                    )

                def v_sub(ci, sub):
                    xkt, off, w = xkts[ci]
                    kcg = (off + sub * 128) // 128
                    v_ps = psW.tile([128, DV], F32, tag="pa")
                    for k in range(DCH):
                        nc.tensor.matmul(
                            v_ps[:, :],
                            xkt[:, k, sub * 128:(sub + 1) * 128],
                            wv_r[:, k, :],
                            start=(k == 0), stop=False,
                        )
                    nc.tensor.matmul(
                        v_ps[:, :], ones_r[0:1, :], bv_r[0:1, :],
                        start=False, stop=True,
                    )
                    nc.vector.tensor_copy(
                        out=vp[:, kcg, :, 0:HD],
                        in_=v_ps.rearrange("p (h d) -> p h d", h=HC),
                    )

                # fill list: v-subs early (AV(h0) needs them all), q groups
                # interleaved ahead of their consuming head
                fills = {}
                vi = 0
                order = []
                for ci, (xkt, off, w) in enumerate(xkts):
                    for sub in range(w // 128):
                        order.append((('v', vi), lambda ci=ci, sub=sub: v_sub(ci, sub)))
                        vi += 1
                for mc in range(1, MC):
                    for lc in range(2):
                        order.append((('q', mc, lc), lambda mc=mc, lc=lc: q_group(mc, lc)))
                for mc in range(MC):
                    for lc in range(2, 4):
                        order.append((('q', mc, lc), lambda mc=mc, lc=lc: q_group(mc, lc)))
                fills = dict(order)

                def drain(n=1):
                    for _ in range(n):
                        if not fills:
                            return
                        k = next(iter(fills))
                        fills.pop(k)()

                def force(*keys):
                    for k in keys:
                        if k in fills:
                            fills.pop(k)()

                # q pair 0, hemi 0 — needed before the first scores
                q_group(0, 0)
                q_group(0, 1)

                # ---------------- attention ----------------
                on_pairs = {}
                pend = {}

                def emit_scores(h, qh):
                    c, par = h // 2, h % 2
                    q0 = qh * 1024
                    pts = []
                    for kc in range(KC):
                        st = psB.tile([128, 1024], F32, tag="st")
                        for s in range(2):
                            nc.tensor.matmul(
                                st[:, s * 512:(s + 1) * 512],
                                kT[64 * par:64 * par + 64, c, kc * 128:(kc + 1) * 128],
                                qT[64 * par:64 * par + 64, c, q0 + s * 512:q0 + (s + 1) * 512],
                                start=True, stop=True,
                            )
                        pt = workP.tile([128, 1024], BF16, tag="pT", bufs=24)
                        nc.scalar.activation(
                            out=pt, in_=st, func=AF.Exp,
                            bias=mb_t[:, kc:kc + 1], scale=1.0,
                        )
                        pts.append(pt)
                        if kc in (1, 3, 5, 7):
                            drain(1)
                    pend[(h, qh)] = pts

                def emit_av(h, qh):
                    c, par = h // 2, h % 2
                    q0 = qh * 1024
                    pts = pend.pop((h, qh))
                    if par == 0:
                        on_pairs[(c, qh)] = [None] * 8
                    for qc in range(8):
                        ot2 = psC.tile([128, 512], F32, tag="ot")
                        for kc in range(KC):
                            nc.tensor.matmul(
                                ot2[:, 0:HD + 1],
                                pts[kc][:, qc * 128:(qc + 1) * 128],
                                vp[:, kc, h, :],
                                start=(kc == 0), stop=(kc == KC - 1),
                            )
                        recip2 = workS.tile([128, 1], F32, tag="rc", bufs=4)
                        nc.vector.reciprocal(out=recip2, in_=ot2[:, HD:HD + 1])
                        if par == 0:
                            onp = workS.tile([128, 128], BF16, tag="on", bufs=16)
                            on_pairs[(c, qh)][qc] = onp
                            tgt = onp[:, 0:HD]
                        else:
                            onp = on_pairs[(c, qh)][qc]
                            tgt = onp[:, HD:128]
                        nc.vector.tensor_scalar_mul(
                            out=tgt, in0=ot2[:, 0:HD], scalar1=recip2,
                        )
                        if par == 1:
                            nc.sync.dma_start(
                                out=otn[:, c, q0 + qc * 128:q0 + (qc + 1) * 128],
                                in_=onp, transpose=True,
                            )

                def emit_outproj(qt):
                    y_sb = workS.tile([128, DOUT], BF16, tag="y")
                    for dc in range(DC):
                        y_ps = psW.tile([128, 512], F32, tag="pa")
                        for pr in range(MC):
                            nc.tensor.matmul(
                                y_ps[:, :],
                                otn[:, pr, qt * 128:(qt + 1) * 128],
                                wo_r[:, pr, dc * 512:(dc + 1) * 512],
                                start=(pr == 0), stop=False,
                            )
                        nc.tensor.matmul(
                            y_ps[:, :], ones_r[0:1, :], bo_r[0:1, dc * 512:(dc + 1) * 512],
                            start=False, stop=True,
                        )
                        nc.vector.tensor_copy(
                            out=y_sb[:, dc * 512:(dc + 1) * 512], in_=y_ps,
                        )
                    nc.sync.dma_start(out=y_d[qt * 128:(qt + 1) * 128, :], in_=y_sb)

                prev = None
                first_av = True
                for qh in range(QH):
                    for h in range(HC):
                        c = h // 2
                        force(('q', c, 2 * qh), ('q', c, 2 * qh + 1))
                        emit_scores(h, qh)
                        drain(1)
                        if prev is not None:
                            if first_av:
                                force(*[('v', i) for i in range(KC)])
                                first_av = False
                            emit_av(*prev)
                            if qh == 1 and h >= 1:
                                emit_outproj(h - 1)
                        prev = (h, qh)
                # tail: last AV, then remaining out-proj tiles
                for k in list(fills):
                    fills.pop(k)()
                emit_av(*prev)
                for qt in range(HC - 1, L // 128):
                    emit_outproj(qt)
    # split multi-waits (walrus allows 1 sync wait per instruction reliably)
    if waitsplit:
        _split_excess_waits(nc)
    return nc


def _plan(mask, L, D, H):
    """Shared cfg incl. padded valid-key count (multiple of 128)."""
    valid = (~np.asarray(mask, bool)).sum(axis=1)
    lv = int(valid.max())
    lv_pad = max(128, min(L, ((lv + 127) // 128) * 128))
    return {"L": L, "D": D, "HC": H // 2, "HD": D // H, "Lv": lv_pad}


def _prep_inputs(x, mask, W_qkv, b_qkv, W_out, b_out, cfg):
    """Build the 8 per-core input maps (host-side shuffles)."""
    import ml_dtypes

    BF = ml_dtypes.bfloat16
    L, D, HC, HD, Lv = cfg["L"], cfg["D"], cfg["HC"], cfg["HD"], cfg["Lv"]
    DV = HC * HD
    MC = DV // 128
    N = x.shape[0]
    scale = 1.0 / np.sqrt(HD)
    Wt = np.ascontiguousarray(W_qkv.T).astype(np.float32)    # [D, 3D]
    WoT = np.ascontiguousarray(W_out.T).astype(np.float32)   # [D, D]
    DCH = D // 128
    KC = Lv // 128

    per_hg = []
    for hg in range(2):
        qs, ks, vs = hg * DV, D + hg * DV, 2 * D + hg * DV
        wqk = np.concatenate(
            [Wt[:, qs:qs + DV] * scale, Wt[:, ks:ks + DV]], axis=1
        )  # [D, 2DV]
        wqk = wqk.reshape(DCH, 128, 2 * DV)
        wqk = np.ascontiguousarray(wqk.transpose(1, 0, 2)).astype(BF)
        wv = Wt[:, vs:vs + DV].reshape(DCH, 128, DV)
        wv = np.ascontiguousarray(wv.transpose(1, 0, 2)).astype(BF)
        bqk = np.concatenate(
            [b_qkv[qs:qs + DV] * scale, b_qkv[ks:ks + DV]]
        ).reshape(2 * MC, 128)
        bqk = np.ascontiguousarray(bqk.T).astype(np.float32)  # [128, 2MC]
        bv = np.ascontiguousarray(b_qkv[vs:vs + DV][None, :]).astype(np.float32)
        # wo: [128, MC, D] — head-pair packed rows (pair pr = heads 2pr,2pr+1)
        wo_heads = WoT[hg * DV:(hg + 1) * DV, :].reshape(HC, HD, D)
        wo = np.ascontiguousarray(
            wo_heads.reshape(MC, 2 * HD, D).transpose(1, 0, 2)
        ).astype(BF)
        per_hg.append(dict(wqk=wqk, wv=wv, bqk=bqk, bv=bv, wo=wo))

    # b_out only on hg=0 cores; partials are summed on host (avoid 2x bias)
    bo_full = np.ascontiguousarray(b_out[None, :]).astype(np.float32)
    bo_zero = np.zeros_like(bo_full)
    xTs, xkTs, mbs = [], [], []
    for n in range(N):
        xTs.append(np.ascontiguousarray(x[n].T).astype(BF))
        kidx = np.nonzero(~np.asarray(mask[n], bool))[0]
        xk = np.zeros((Lv, D), np.float32)
        xk[:len(kidx)] = x[n][kidx]
        xkTs.append(np.ascontiguousarray(xk.T).astype(BF))
        mb = np.full(Lv, -1e9, np.float32)
        mb[:len(kidx)] = 0.0
        mbs.append(np.ascontiguousarray(mb.reshape(KC, 128).T))

    in_maps = []
    for c in range(2 * N):
        n, hg = c // 2, c % 2
        d = dict(per_hg[hg])
        d.update(xT=xTs[n], xkT=xkTs[n], mb=mbs[n],
                 bo=(bo_full if hg == 0 else bo_zero))
        in_maps.append(d)
    return in_maps


def kernel(x, mask, W_qkv, b_qkv, W_out, b_out):
    from concourse.bass_utils import run_bass_kernel_spmd

    x = np.asarray(x, dtype=np.float32)
    mask = np.asarray(mask)
    N, L, D = x.shape
    H = 16
    cfg = _plan(mask, L, D, H)

    key = (L, D, H, cfg["Lv"])
    if key not in _KERNEL_CACHE:
        _KERNEL_CACHE[key] = _build(cfg)
    nc = _KERNEL_CACHE[key]

    in_maps = _prep_inputs(
        x, mask,
        np.asarray(W_qkv, np.float32), np.asarray(b_qkv, np.float32),
        np.asarray(W_out, np.float32), np.asarray(b_out, np.float32), cfg,
    )
    res = run_bass_kernel_spmd(nc, in_maps, list(range(2 * N)))
    out = np.empty((N, L, D), np.float32)
    for n in range(N):
        out[n] = (np.asarray(res.results[2 * n]["y"]).astype(np.float32)
                  + np.asarray(res.results[2 * n + 1]["y"]).astype(np.float32))
    return out
